# revision 103
# baseline (speedup 1.0000x reference)
"""BailingMoE linear attention block on 8 trn2 cores (tensor-parallel over heads).

Per rank r of 8: owns heads 2r, 2r+1 (256 of 2048 features). Single fused pass
over 8 super-chunks of 512 tokens: qkv+gate projections (bf16 matmuls, q/k/gate
feature-major [d,t], v directly in natural [t,d] layout), RoPE, chunked linear
attention (B=128), gated output, and the row-slice of w_dense -> partial y in
natural layout, all pipelined so the PE never waits on a phase boundary.
Rank also emits per-head sum-of-squares of o per token (ssq). Host combines:
y = (sum_r y_r) * rsqrt(mean(ssq)/HD + eps) row-scale (RMSNorm commutes through
the dense matmul).

All matmul operands are bf16 (1 cycle/row at any moving size, halves DMA);
PSUM accumulation stays fp32. PSUM budget: proj 2 + scores/transpose/state/ssq
2 + o 2 + dense 2 = 8 banks. Startup runs the first projection k-major over 6
accumulators interleaved with 2KB-descriptor weight/activation DMAs; dense
output is staged [128, 2048] bf16 per row-tile (one DMA each).
"""
import numpy as np
import ml_dtypes
import concourse.bass as bass
import concourse.mybir as mybir
import concourse.tile as tile
from concourse import bacc, bass_utils

T, HID, H, D = 4096, 2048, 16, 128
EPS = 1e-5
THETA = 600000.0
NUM_LAYERS, LAYER_ID = 32, 0
M = 8                 # cores
HPR = H // M          # heads per rank = 2
CW = HPR * D          # feature columns per rank = 256
B = 128               # attention chunk
SC = 512              # super-chunk (projection slice)
JH = B // 128         # 128-row j-subtiles per chunk
CPS = SC // B         # chunks per super-chunk
NCH = T // B          # chunks
NSC = T // SC         # 8
NK = HID // 128       # 16 k-tiles

f32 = mybir.dt.float32
bf16 = mybir.dt.bfloat16

_PROGRAM = None


def _build_program():
    nc = bacc.Bacc(trn_type="TRN2")

    hT = nc.dram_tensor("hT", [HID, T], bf16, kind="ExternalInput")
    w_all = nc.dram_tensor("w_all", [HID, 4 * CW], bf16, kind="ExternalInput")
    wd = nc.dram_tensor("wd", [CW, HID], bf16, kind="ExternalInput")
    cosT = nc.dram_tensor("cosT", [D, T], f32, kind="ExternalInput")
    sinT = nc.dram_tensor("sinT", [D, T], f32, kind="ExternalInput")
    maskT = nc.dram_tensor("maskT", [128, HPR * JH * B], bf16, kind="ExternalInput")
    qdtab = nc.dram_tensor("qdtab", [128, HPR * B], bf16, kind="ExternalInput")
    kdcol = nc.dram_tensor("kdcol", [128, HPR * JH], f32, kind="ExternalInput")
    bdcol = nc.dram_tensor("bdcol", [128, HPR], f32, kind="ExternalInput")

    y_nat = nc.dram_tensor("y_nat", [T, HID], bf16, kind="ExternalOutput")
    ssq = nc.dram_tensor("ssq", [NCH // 2, HPR * 2 * B], f32,
                         kind="ExternalOutput")

    half = D // 2

    with tile.TileContext(nc) as tc:
        with tc.tile_pool(name="persist", bufs=1) as persist:
            w_sb = persist.tile([128, NK, 4 * CW], bf16, name="w_sb")
            wd_sb = persist.tile([128, HPR, HID], bf16, name="wd_sb")
            mk_sb = persist.tile([128, HPR * JH * B], bf16, name="mk_sb")
            qd_sb = persist.tile([128, HPR * B], bf16, name="qd_sb")
            kd_sb = persist.tile([128, HPR * JH], f32, name="kd_sb")
            bd_sb = persist.tile([128, HPR], f32, name="bd_sb")
            ident = persist.tile([128, 128], bf16, name="ident")
            ident_f = persist.tile([128, 128], f32, name="ident_f")
            ones_col = persist.tile([128, 1], bf16, name="ones_col")
            ones_f = persist.tile([128, 1], f32, name="ones_f")
            S = persist.tile([128, HPR, 128], bf16, name="S")
            zero_f = persist.tile([128, HPR, 128], f32, name="zero_f")

            from concourse.masks import make_identity

            make_identity(nc, ident_f[:])
            nc.vector.tensor_copy(ident[:], ident_f[:])
            warm = persist.tile([128, 128], bf16, name="warm")
            nc.vector.tensor_copy(warm[:], ident_f[:])
            nc.gpsimd.memset(ones_f[:], 1.0)
            nc.vector.tensor_copy(ones_col[:], ones_f[:])
            nc.gpsimd.memset(zero_f[:], 0.0)
            nc.vector.tensor_copy(S[:], zero_f[:])

            with (
                tc.tile_pool(name="hkp", bufs=2) as hkp,
                tc.tile_pool(name="scp", bufs=2) as scp,
                tc.tile_pool(name="ropep", bufs=3) as ropep,
                tc.tile_pool(name="workp", bufs=6) as workp,
                tc.tile_pool(name="outp", bufs=3) as outp,
                tc.tile_pool(name="psP", bufs=2, space="PSUM") as psP,
                tc.tile_pool(name="psA", bufs=2, space="PSUM") as psA,
                tc.tile_pool(name="psO", bufs=2, space="PSUM") as psO,
                tc.tile_pool(name="psD", bufs=2, space="PSUM") as psD,
            ):
                # startup DMA order: interleave full weight rows (2 KB
                # descriptors run at ~2x the DMA rate of 1 KB ones) with the
                # first hk PAIR (two super-chunks of hidden state per fetch,
                # also 2 KB descriptors) so the PE's first projection group
                # starts ~1 us in and sc1's data arrives for free.
                hk0 = hkp.tile([128, NK, 2 * SC], bf16, tag="hk", name="hkp0")
                cs0 = ropep.tile([128, SC], f32, tag="cs", name="cs0")
                sn0 = ropep.tile([128, SC], f32, tag="sn", name="sn0")
                for k in range(NK):
                    nc.sync.dma_start(
                        w_sb[:, k, :], w_all[k * 128:(k + 1) * 128, :]
                    )
                    nc.sync.dma_start(
                        hk0[:, k, :], hT[k * 128:(k + 1) * 128, 0:2 * SC]
                    )
                nc.sync.dma_start(cs0[:], cosT[:, 0:SC])
                nc.sync.dma_start(sn0[:], sinT[:, 0:SC])
                nc.sync.dma_start(mk_sb[:], maskT[:, :])
                nc.sync.dma_start(qd_sb[:], qdtab[:, :])
                nc.sync.dma_start(kd_sb[:], kdcol[:, :])
                nc.sync.dma_start(bd_sb[:], bdcol[:, :])
                for kf in range(HPR):
                    nc.sync.dma_start(wd_sb[:, kf, :],
                                      wd[kf * 128:(kf + 1) * 128, :])

                # warm the PE clock gate during the startup DMA wait:
                # ~3 us of dependency-free matmuls bring the HAM to full
                # rate before the first real projection arrives
                warm_ps = psD.tile([128, 128], f32, tag="d", name="warm_ps")
                for i in range(20):
                    nc.tensor.matmul(warm_ps[:], warm[:], warm[:],
                                     start=True, stop=True)

                hk_pair = hk0

                def emit_dense(c, xc, extra_pool=None, split_dma=False,
                               act_copies=False):
                    for tsub in range(B // 128):
                        dsb = outp.tile([128, HID], bf16, tag="dsb",
                                        name=f"db{c}_{tsub}")
                        for ms in range(4):
                            g = tsub * 4 + ms
                            if extra_pool is not None and g % 2 == 1:
                                dps = extra_pool.tile(
                                    [128, 512], f32, tag="acc",
                                    name=f"d{c}_{tsub}_{ms}")
                            else:
                                dps = psD.tile([128, 512], f32, tag="d",
                                               name=f"d{c}_{tsub}_{ms}")
                            for kf in range(HPR):
                                nc.tensor.matmul(
                                    dps[:],
                                    xc[:, kf, tsub * 128:(tsub + 1) * 128],
                                    wd_sb[:, kf, ms * 512:(ms + 1) * 512],
                                    start=(kf == 0),
                                    stop=(kf == HPR - 1),
                                )
                            with tc.high_priority(offset=64):
                                if g % 2 == 0 and not act_copies:
                                    nc.vector.tensor_copy(
                                        dsb[:, ms * 512:(ms + 1) * 512], dps[:]
                                    )
                                else:
                                    nc.scalar.copy(
                                        dsb[:, ms * 512:(ms + 1) * 512], dps[:]
                                    )
                            if split_dma and ms % 2 == 1:
                                nc.sync.dma_start(
                                    y_nat[(c * (B // 128) + tsub) * 128:
                                          (c * (B // 128) + tsub + 1) * 128,
                                          (ms - 1) * 512:(ms + 1) * 512],
                                    dsb[:, (ms - 1) * 512:(ms + 1) * 512],
                                )
                        if not split_dma:
                            nc.sync.dma_start(
                                y_nat[(c * (B // 128) + tsub) * 128:
                                      (c * (B // 128) + tsub + 1) * 128, :],
                                dsb[:],
                            )

                for sc in range(NSC):
                    tsl = bass.ds(sc * SC, SC)
                    po = (sc % 2) * SC          # offset within the hk pair
                    if sc % 2 == 0 and sc > 0:
                        hk_pair = hkp.tile([128, NK, 2 * SC], bf16, tag="hk",
                                           name=f"hkp{sc}")
                        for k in range(NK):
                            nc.sync.dma_start(
                                hk_pair[:, k, :],
                                hT[k * 128:(k + 1) * 128,
                                   sc * SC:(sc + 2) * SC],
                            )
                    hk = hk_pair
                    if sc == 0:
                        cs, sn = cs0, sn0
                    else:
                        cs = ropep.tile([128, SC], f32, tag="cs", name=f"cs{sc}")
                        nc.sync.dma_start(cs[:], cosT[:, tsl])
                        sn = ropep.tile([128, SC], f32, tag="sn", name=f"sn{sc}")
                        nc.sync.dma_start(sn[:], sinT[:, tsl])

                    qsb = scp.tile([128, HPR, SC], bf16, tag="q", name=f"q{sc}")
                    ksb = scp.tile([128, HPR, SC], bf16, tag="k", name=f"k{sc}")
                    gsb = scp.tile([128, HPR, SC], bf16, tag="g", name=f"g{sc}")
                    vnb = scp.tile([128, 4, CW], bf16, tag="v", name=f"v{sc}")

                    # Projection group order q0,q1,v0..v3,k0,k1,g0,g1: the
                    # slow rope drains (3 DVE reads) land against long next
                    # fills, the fast copy/sigmoid drains against short ones,
                    # so the 2-slot PSUM rotation never blocks the PE.
                    def fm_group(col, idx):
                        acc = psP.tile(
                            [128, SC], f32, tag="acc", name=f"acc{sc}_{idx}"
                        )
                        for k in range(NK):
                            nc.tensor.matmul(
                                acc[:],
                                w_sb[:, k, col:col + 128],
                                hk[:, k, po:po + SC],
                                start=(k == 0),
                                stop=(k == NK - 1),
                            )
                        return acc

                    def rope_drain(acc, dst, hh, idx):
                        nm = f"r{sc}_{idx}"
                        # one fast ACT read evacuates the PSUM slot; the DVE
                        # rope chain then runs off SBUF at its own pace
                        af = ropep.tile([128, SC], f32, tag="af", name=f"af{nm}")
                        nc.scalar.copy(af[:], acc[:])
                        t1 = ropep.tile([128, SC], bf16, tag="t1", name=f"t1{nm}")
                        tmp = ropep.tile([128, SC], bf16, tag="tp", name=f"tp{nm}")
                        nc.vector.tensor_mul(t1[:], af[:], cs[:])
                        nc.vector.tensor_mul(
                            tmp[0:half, :], af[half:D, :], sn[half:D, :]
                        )
                        nc.vector.tensor_mul(
                            tmp[half:D, :], af[0:half, :], sn[0:half, :]
                        )
                        nc.vector.tensor_add(dst[:, hh, :], t1[:], tmp[:])

                    if sc == 0:
                        # k-major over 6 parallel accumulators (2 psP + 2 psA
                        # + 2 psO banks): each k-tile arriving from the
                        # startup DMA stream feeds 6 matmuls (1.28 us of PE
                        # work per 1.25 us DMA cadence), so the PE barely
                        # trails the startup weight/activation stream.
                        accs = [
                            psP.tile([128, SC], f32, tag="acc", name=f"acq{i}")
                            for i in range(2)
                        ] + [
                            psA.tile([128, SC], f32, tag="pa", name=f"acq{i+2}")
                            for i in range(2)
                        ] + [
                            psO.tile([128, SC], f32, tag="o", name=f"acq{i+4}")
                            for i in range(2)
                        ]
                        # two vnat groups ride along k-major in the two
                        # spare psD banks: 6*213 + 2*107 = 1492 ns of PE work
                        # per 1456 ns DMA pair cadence, so the PE no longer
                        # trails the startup stream
                        accv01 = [
                            psD.tile([128, CW], f32, tag="d", name=f"av0_{i}")
                            for i in range(2)
                        ]
                        cols = [0, 128, CW, CW + 128, 3 * CW, 3 * CW + 128]
                        for k in range(NK):
                            for i in range(6):
                                nc.tensor.matmul(
                                    accs[i][:],
                                    w_sb[:, k, cols[i]:cols[i] + 128],
                                    hk[:, k, po:po + SC],
                                    start=(k == 0),
                                    stop=(k == NK - 1),
                                )
                            for i in range(2):
                                nc.tensor.matmul(
                                    accv01[i][:],
                                    hk[:, k, po + i * 128:po + (i + 1) * 128],
                                    w_sb[:, k, 2 * CW:3 * CW],
                                    start=(k == 0),
                                    stop=(k == NK - 1),
                                )
                        for i in range(2):
                            with tc.high_priority(offset=64):
                                nc.scalar.copy(vnb[:, i, :], accv01[i][:])
                        # interleave rope drains with the remaining vnat
                        # groups so the ACT queue alternates slot-freeing
                        # copies and the psP rotation never starves
                        rd = [(accs[0], qsb, 0, 0), (accs[1], qsb, 1, 1),
                              (accs[2], ksb, 0, 2), (accs[3], ksb, 1, 3)]
                        for tsub in range(4):
                            rope_drain(*rd[tsub])
                            if tsub >= 2:
                                continue
                            vt = 2 + tsub
                            accv = psP.tile(
                                [128, CW], f32, tag="acc", name=f"av0_{vt}"
                            )
                            for k in range(NK):
                                nc.tensor.matmul(
                                    accv[:],
                                    hk[:, k, po + vt * 128:
                                       po + (vt + 1) * 128],
                                    w_sb[:, k, 2 * CW:3 * CW],
                                    start=(k == 0),
                                    stop=(k == NK - 1),
                                )
                            with tc.high_priority(offset=64):
                                nc.scalar.copy(vnb[:, vt, :], accv[:])
                        for hh in range(HPR):
                            nc.scalar.activation(
                                gsb[:, hh, :], accs[4 + hh][:],
                                mybir.ActivationFunctionType.Sigmoid,
                            )
                    else:
                        for hh in range(HPR):                   # q
                            acc = fm_group(hh * 128, hh)
                            rope_drain(acc, qsb, hh, hh)
                        for hh in range(HPR):                   # k
                            acc = fm_group(CW + hh * 128, 2 + hh)
                            rope_drain(acc, ksb, hh, 2 + hh)
                    if sc > 0:
                        for tsub in range(4):                   # v (natural)
                            accv = psP.tile(
                                [128, CW], f32, tag="acc", name=f"av{sc}_{tsub}"
                            )
                            for k in range(NK):
                                nc.tensor.matmul(
                                    accv[:],
                                    hk[:, k,
                                       po + tsub * 128:po + (tsub + 1) * 128],
                                    w_sb[:, k, 2 * CW:3 * CW],
                                    start=(k == 0),
                                    stop=(k == NK - 1),
                                )
                            nc.scalar.copy(vnb[:, tsub, :], accv[:])
                        for hh in range(HPR):                   # gate
                            acc = fm_group(3 * CW + hh * 128, 4 + hh)
                            nc.scalar.activation(
                                gsb[:, hh, :], acc[:],
                                mybir.ActivationFunctionType.Sigmoid,
                            )

                    # ---- k natural (PE transpose), decay-scaled: the
                    # decay scale only depends on the head (and j-subtile),
                    # so all CPS chunks batch into one PSUM tile and one
                    # scaled ACT copy per head ----
                    knb = {}
                    for h in range(HPR):
                        tvb = psO.tile([128, CPS * JH, 128], bf16, tag="o",
                                       name=f"tvb{sc}_{h}")
                        for cj in range(CPS * JH):
                            nc.tensor.transpose(
                                tvb[:, cj, :],
                                ksb[:, h, cj * 128:(cj + 1) * 128],
                                ident[:],
                            )
                        knt = workp.tile([128, CPS * JH, 128], bf16,
                                         tag=f"knb{h}", name=f"knb{sc}_{h}")
                        nc.scalar.activation(
                            knt[:], tvb[:],
                            mybir.ActivationFunctionType.Copy,
                            scale=kd_sb[:, h * JH:h * JH + 1],
                        )
                        knb[h] = knt

                    # ---- attention, CPS chunks of B tokens.  On the
                    # last super-chunk, all chunks' score matmuls + masks
                    # run as a front-loaded phase so the DVE mask work
                    # clears before the tail dense copies crowd its queue.
                    def emit_scores(hf):
                        c = sc * CPS + hf
                        i0 = hf * B
                        nm = f"c{c}"
                        m1 = {}
                        for h in range(HPR):
                            qs = qsb[:, h, i0:i0 + B]
                            for jh in range(JH):
                                pt = psA.tile([128, B], f32, tag="pa",
                                              name=f"pt{nm}_{h}_{jh}")
                                nc.tensor.matmul(
                                    pt[:],
                                    ksb[:, h,
                                        i0 + jh * 128:i0 + (jh + 1) * 128],
                                    qs,
                                    start=True, stop=True,
                                )
                                m = workp.tile([128, B], bf16,
                                               tag=f"m{jh}_{hf % 2}",
                                               name=f"m{nm}_{h}_{jh}")
                                nc.vector.tensor_mul(
                                    m[:], pt[:],
                                    mk_sb[:, (h * JH + jh) * B:
                                          (h * JH + jh + 1) * B],
                                )
                                m1[(h, jh)] = m
                        qp = {}
                        for h in range(HPR):
                            qpt = workp.tile([128, B], bf16,
                                             tag=f"qp{h}_{hf % 2}",
                                             name=f"qp{nm}_{h}")
                            nc.gpsimd.tensor_mul(
                                qpt[:], qsb[:, h, i0:i0 + B],
                                qd_sb[:, h * B:(h + 1) * B],
                            )
                            qp[h] = qpt
                        return m1, qp

                    xc_of = {}
                    scores_of = {}
                    if sc == NSC - 1:
                        for hf in range(CPS):
                            scores_of[hf] = emit_scores(hf)
                    for hf in range(CPS):
                        c = sc * CPS + hf
                        i0 = hf * B
                        nm = f"c{c}"
                        xc = workp.tile([128, HPR, B], bf16, tag="xc",
                                        name=f"xc{nm}")
                        xc_of[hf] = xc
                        if hf % 2 == 0:
                            sqh2 = workp.tile([128, HPR, 2 * B], bf16,
                                              tag="sqh", name=f"sqh{nm}")
                        sqh = sqh2[:, :, (hf % 2) * B:(hf % 2) * B + B]
                        if sc == NSC - 1:
                            m1, qp = scores_of[hf]
                        else:
                            m1, qp = emit_scores(hf)
                        kn = {(h, jh): knb[h][:, hf * JH + jh, :]
                              for h in range(HPR) for jh in range(JH)}
                        # o = V^T (mask*scores) + S^T q'
                        for h in range(HPR):
                            o = psO.tile([128, B], f32, tag="o",
                                         name=f"o{nm}_{h}")
                            for jh in range(JH):
                                nc.tensor.matmul(
                                    o[:],
                                    vnb[:, hf * JH + jh, h * 128:(h + 1) * 128],
                                    m1[(h, jh)][:], start=(jh == 0), stop=False,
                                )
                            nc.tensor.matmul(
                                o[:], S[:, h, :], qp[h][:],
                                start=False, stop=True,
                            )
                            nc.scalar.square(sqh[:, h, :], o[:])
                            # state update S = bd*S + K'^T V; the stt goes
                            # into the DVE queue before the gate multiply so
                            # the next chunk's state matmul unblocks sooner
                            sps = psA.tile([128, 128], f32, tag="pa",
                                           name=f"sp{nm}_{h}")
                            for jh in range(JH):
                                nc.tensor.matmul(
                                    sps[:], kn[(h, jh)],
                                    vnb[:, hf * JH + jh, h * 128:(h + 1) * 128],
                                    start=(jh == 0), stop=(jh == JH - 1),
                                )
                            nc.vector.scalar_tensor_tensor(
                                out=S[:, h, :],
                                in0=S[:, h, :],
                                scalar=bd_sb[:, h:h + 1],
                                in1=sps[:],
                                op0=mybir.AluOpType.mult,
                                op1=mybir.AluOpType.add,
                            )
                            nc.vector.tensor_mul(
                                xc[:, h, :], o[:], gsb[:, h, i0:i0 + B]
                            )
                        # ssq: per-head column sums of o^2, once per
                        # chunk pair
                        if hf % 2 == 1:
                            sqp = psA.tile([1, HPR, 2 * B], f32, tag="pa",
                                           name=f"sq{nm}")
                            nc.tensor.matmul(
                                sqp[:], ones_col[:], sqh2[:],
                                start=True, stop=True,
                            )
                            sqs = outp.tile([1, HPR * 2 * B], f32, tag="sqs",
                                            name=f"sqs{nm}")
                            for h in range(HPR):
                                nc.scalar.copy(
                                    sqs[:, h * 2 * B:(h + 1) * 2 * B],
                                    sqp[:, h, :],
                                )
                            nc.sync.dma_start(ssq[c // 2:c // 2 + 1, :],
                                              sqs[:])
                        if sc == NSC - 1:
                            # run one chunk behind: the previous chunk's
                            # dense fills this tail attention's handoff
                            # stalls; only the final chunk's dense output
                            # remains to drain at the very end
                            if hf == 0:
                                emit_dense(deferred[0], deferred[1],
                                           extra_pool=psP)
                            else:
                                emit_dense(c - 1, xc_of[hf - 1],
                                           extra_pool=psP,
                                           split_dma=(hf == CPS - 1))

                    # ---- fused dense after the chunks' attention ----
                    if sc == NSC - 2:
                        for hf in range(CPS - 1):
                            emit_dense(sc * CPS + hf, xc_of[hf])
                        deferred = (sc * CPS + CPS - 1, xc_of[CPS - 1])
                    elif sc == NSC - 1:
                        emit_dense(sc * CPS + CPS - 1, xc_of[CPS - 1],
                                   extra_pool=psP, split_dma=True)
                    else:
                        for hf in range(CPS):
                            emit_dense(sc * CPS + hf, xc_of[hf])

    nc.compile()
    return nc


def _slopes(n):
    start = 2.0 ** (-(2.0 ** -(np.log2(n) - 3)))
    return np.array([start ** (i + 1) for i in range(n)], dtype=np.float64)


def kernel(hidden_states, positions, w_qkv, w_g, w_dense, g_norm_weight):
    global _PROGRAM
    if _PROGRAM is None:
        _PROGRAM = _build_program()
    nc = _PROGRAM

    bf = ml_dtypes.bfloat16
    hidden_states = np.asarray(hidden_states, dtype=np.float32)
    positions = np.asarray(positions)
    w_qkv = np.asarray(w_qkv, dtype=np.float32)
    w_g = np.asarray(w_g, dtype=np.float32)
    w_dense = np.asarray(w_dense, dtype=np.float32)
    g_norm_weight = np.asarray(g_norm_weight, dtype=np.float32)

    hT = np.ascontiguousarray(hidden_states.T).astype(bf)

    # rope tables, feature-major; sinT carries the rotate-half signs
    half = D // 2
    inv_freq = 1.0 / (THETA ** (np.arange(0, D, 2, dtype=np.float64) / D))
    freqs = positions.astype(np.float64)[:, None] * inv_freq          # [T, 64]
    cos = np.cos(freqs).T                                             # [64, T]
    sin = np.sin(freqs).T
    cosT = np.concatenate([cos, cos], axis=0).astype(np.float32)      # [128, T]
    sinT = np.concatenate([sin, -sin], axis=0).astype(np.float32)

    s = _slopes(H) * (1.0 - LAYER_ID / (NUM_LAYERS - 1) + 1e-5)       # [16]
    idx = np.arange(B, dtype=np.float64)
    diff = idx[:, None] - idx[None, :]
    scale = D ** -0.5
    decay = np.where(
        diff[None, :, :] >= 0, np.exp(-s[:, None, None] * diff[None, :, :]), 0.0
    )                                                                  # [16, B, B]
    qd = np.exp(-s[:, None] * (idx[None, :] + 1.0)) * scale            # [16, B]
    kd = np.exp(-s[:, None] * (B - 1.0 - idx[None, :]))                # [16, B]
    bd = np.exp(-s * B)                                                # [16]

    in_maps = []
    for r in range(M):
        heads = [HPR * r + i for i in range(HPR)]
        cols = slice(r * CW, (r + 1) * CW)
        wq = w_qkv[:, r * CW:(r + 1) * CW]
        wk = w_qkv[:, HID + r * CW: HID + (r + 1) * CW]
        wv = w_qkv[:, 2 * HID + r * CW: 2 * HID + (r + 1) * CW]
        wg = w_g[:, cols]
        w_all = np.concatenate([wq, wk, wv, wg], axis=1)               # [HID, 4*CW]
        wdr = (g_norm_weight[cols, None] * w_dense[cols, :])

        mk = np.empty((128, HPR * JH * B), np.float32)
        qdt = np.empty((128, HPR * B), np.float32)
        kdc = np.empty((128, HPR * JH), np.float32)
        bdc = np.empty((128, HPR), np.float32)
        for i, h in enumerate(heads):
            mTh = (decay[h].T * scale)                                 # [j, i]
            for jh in range(JH):
                mk[:, (i * JH + jh) * B:(i * JH + jh + 1) * B] = (
                    mTh[jh * 128:(jh + 1) * 128, :]
                )
                kdc[:, i * JH + jh] = kd[h, jh * 128:(jh + 1) * 128]
            qdt[:, i * B:(i + 1) * B] = np.broadcast_to(
                qd[h][None, :], (128, B)
            )
            bdc[:, i] = bd[h]

        in_maps.append(
            {
                "hT": hT,
                "w_all": np.ascontiguousarray(w_all).astype(bf),
                "wd": np.ascontiguousarray(wdr).astype(bf),
                "cosT": cosT,
                "sinT": sinT,
                "maskT": mk.astype(bf),
                "qdtab": qdt.astype(bf),
                "kdcol": kdc,
                "bdcol": bdc,
            }
        )

    global _LAST_IN_MAPS
    _LAST_IN_MAPS = in_maps
    results = bass_utils.run_bass_kernel_spmd(nc, in_maps, core_ids=list(range(M)))

    y_sum = np.zeros((T, HID), np.float64)
    ssq_tot = np.zeros((T,), np.float64)
    for r in range(M):
        y_sum += results.results[r]["y_nat"].astype(np.float64)
        sq = results.results[r]["ssq"].astype(np.float64)
        ssq_tot += sq.reshape(NCH // 2, HPR, 2 * B).sum(axis=1).reshape(T)
    var = ssq_tot / (H * D)
    F = 1.0 / np.sqrt(var + EPS)
    y = y_sum * F[:, None]
    return y.astype(np.float32)
